# revision 42
# baseline (speedup 1.0000x reference)
"""DirectAU loss kernel for Trainium2, SPMD over 8 NeuronCores.

Math (see reference):
  user_e = user_table[user_id]; pos_e = item_table[pos_id]   (B=8192, D=64)
  align  = mean_i ||un_i - pn_i||^2 = 2 - (2/B) sum_i <un_i, pn_i>
  unif(x)= log( (sum_{i<j} exp(-4 + 4 <xn_i, xn_j>)) / npairs )
  out    = align + 0.5*(unif(user_e) + unif(pos_e))

Strategy (v4 pipeline + host-side align):
  - Work split: cores 0-3 the user-table Gram, 4-7 the pos-table
    one; both tables concatenated so the SPMD program is identical and the
    choice lives in the int32 gather indices. Each core owns two adjacent
    1024-row chunk assignments {a1, a1+1} of its table and gathers chunks
    a1..a1+5.
  - The align term is 0.01% of the FLOPs but costs 8 of the 56 gather
    bands (~9us of SWDGE issue on the pacing GpSimd queue); it is folded
    into the host finalization (which already applies the closed-form
    log / diagonal corrections) in float64.
  - Gram matmuls run in fp8-e4m3 with MatmulPerfMode.DoubleRow (0.5
    cycles/row, 2x bf16): normalized rows are cast to fp8 (DVE), PE-
    transposed per 32-dim half into [32, 2, 512]-packed k-tile layout
    (dims 0-31 in k-tile 0, 32-63 in k-tile 1), and copied PSUM->SBUF.
  - Exp bias is uniform (-4) for every tile: diagonal chunks are computed
    strict-upper-triangular at 128-tile granularity, and the 8 self tiles
    [rt,rt] of each diag chunk go to a separate accumulator column; the
    host removes the double-count closed-form ((S_self - B)/2). This packs
    all off-diagonal exp work into 2048-wide ACTIVATEs regardless of block
    structure, minimizing the ~520ns/instr ACT overhead.
  - ACT (scalar engine) exp at 0.833ns/col is the pacer; emission orders
    PE/ACT/DVE queues explicitly (order-only deps) so the in-order engines
    never reorder into stall-prone sequences.
  - Host sums the 8x[128,35] partials and applies the closed-form
    log/align finalization.
"""

import math

import numpy as np

import concourse.bacc as bacc
import concourse.bass as bass
import concourse.mybir as mybir
import concourse.tile as tile
from concourse import bass_utils
from concourse.masks import make_identity
from concourse.tile_rust import add_dep_helper

B = 8192
DIM = 64
NROWS = 100000
NCORES = 8
CHUNK = 1024
NCHUNK = 6  # gathered main chunks per core (C0..C5)
MAIN_BANDS = NCHUNK * 8  # 48
NBAND = MAIN_BANDS  # 48 gather bands (align is host-side)
NGRP = NCHUNK * 2  # 12 transpose groups of 4 bands (512 rows)
F32 = mybir.dt.float32
F8 = mybir.dt.float8e4
I32 = mybir.dt.int32

# accumulator columns: q in {0,1}: col q*17 = self-tile sums of diag chunk q,
# cols q*17+1 .. q*17+16 = off-diagonal exp sums
SELF_COLS = (0, 17)
ACC_W = 42
PSW = 2048  # PSUM work tile width (fp32)

# Schraudolph fast-exp for exp(4s-4): i32 = s*A + B_, bitcast f32. B_ is
# calibrated (C=-480000) to ~2e-4 mean bias over <xn_i,xn_j> ~ N(0,1/64);
# used only for late drain tiles handed to the otherwise-idle DVE.
A_SCH = float(np.float32(4.0 * (2.0 ** 23) / math.log(2.0)))
B_SCH = float(np.float32(127 * 2 ** 23 - 4.0 * (2.0 ** 23) / math.log(2.0) - 480000.0))


def _emit_rsqrt(nc, pool, x_ap, out_ap, n, tag):
    """out = 1/sqrt(x) on the vector engine (bit-hack seed + 3 Newton steps)."""
    MAGIC = 0x5F3759DF
    op = mybir.AluOpType
    ti = pool.tile([128, n], I32, tag=f"{tag}_ti", name=f"{tag}_ti")
    nc.vector.tensor_scalar(
        out=ti[:], in0=x_ap.bitcast(I32), scalar1=1, scalar2=None,
        op0=op.logical_shift_right,
    )
    yi = pool.tile([128, n], I32, tag=f"{tag}_yi", name=f"{tag}_yi")
    # MAGIC - t == (t ^ -1) + (MAGIC + 1); split: ISA can't mix bitwise+arith
    nc.vector.tensor_scalar(
        out=yi[:], in0=ti[:], scalar1=-1, scalar2=None, op0=op.bitwise_xor
    )
    nc.vector.tensor_scalar(
        out=yi[:], in0=yi[:], scalar1=MAGIC + 1, scalar2=None, op0=op.add
    )
    xh = pool.tile([128, n], F32, tag=f"{tag}_xh", name=f"{tag}_xh")
    nc.vector.tensor_scalar(
        out=xh[:], in0=x_ap, scalar1=-0.5, scalar2=None, op0=op.mult
    )
    cur = yi[:].bitcast(F32)
    for it in range(3):
        t2 = pool.tile([128, n], F32, tag=f"{tag}_t2", name=f"{tag}_t2")
        nc.vector.tensor_mul(out=t2[:], in0=cur, in1=cur)
        nc.vector.tensor_mul(out=t2[:], in0=t2[:], in1=xh[:])
        nc.vector.tensor_scalar(
            out=t2[:], in0=t2[:], scalar1=1.5, scalar2=None, op0=op.add
        )
        if it == 2:
            dst_ap = out_ap
        else:
            yt = pool.tile([128, n], F32, tag=f"{tag}_y", name=f"{tag}_y{it}")
            dst_ap = yt[:]
        nc.vector.tensor_mul(out=dst_ap, in0=cur, in1=t2[:])
        cur = dst_ap
    return cur


def _body(tc, tabs, gidx, acc):
    nc = tc.nc
    op = mybir.AluOpType
    DR = mybir.MatmulPerfMode.DoubleRow
    with (
        tc.tile_pool(name="persist", bufs=1) as P,
        tc.tile_pool(name="work", bufs=2) as W,
        tc.tile_pool(name="ps", bufs=2, space="PSUM") as PS,
    ):
        idx_sb = P.tile([128, NBAND], I32, tag="idx")
        nc.sync.dma_start(out=idx_sb[:], in_=gidx)

        accw = P.tile([128, ACC_W], F32, tag="accw")
        nc.vector.memset(accw[:], 0.0)
        bias_o = P.tile([128, 1], F32, tag="bias_o")
        ident = P.tile([128, 128], F32, tag="ident")
        ident8 = P.tile([128, 128], F8, tag="ident8")

        # gathered rows, [128, band, DIM] band-major slots (row c*128+p)
        gath = P.tile([128, NBAND * DIM], F32, tag="gath")
        gath8 = P.tile([128, MAIN_BANDS * DIM], F8, tag="gath8")
        # fp8 transposed layout: group g (4 bands = 512 rows) occupies cols
        # [g*1024, (g+1)*1024): [32 partitions, k-half h in {0,1}, 512 rows];
        # dim d of row r lives at partition d%32, half d//32, col r.
        xnT8 = P.tile([32, NGRP * 1024], F8, tag="xnT8")
        nsq = P.tile([128, NBAND], F32, tag="nsq")
        rinv = P.tile([128, NBAND], F32, tag="rinv")

        # Queue-order pinning: the scheduler's cost model mis-predicts gather
        # and PE readiness, which otherwise reorders the in-order engine
        # queues into stall-prone sequences. Chain DVE normalize stages, and
        # pin the PE and ACT queues to emission order with order-only deps.
        last_dve = [None]
        last_pe = [None]
        last_act = [None]

        def pe_order(inst):
            if last_pe[0] is not None:
                add_dep_helper(inst.ins, last_pe[0].ins, sync=False,
                               reason="pe order")
            last_pe[0] = inst

        def act_order(inst):
            if last_act[0] is not None:
                add_dep_helper(inst.ins, last_act[0].ins, sync=False,
                               reason="act order")
            last_act[0] = inst

        def gather_chunk(ch):
            # NOTE: one indirect DMA per 128-row band. Batching several offset
            # columns into one issue generates corrupt descriptors on HW
            # (wrong rows, byte-misaligned payloads) — do not batch.
            for c in range(ch * 8, (ch + 1) * 8):
                nc.gpsimd.indirect_dma_start(
                    out=gath[:, c * DIM : (c + 1) * DIM],
                    out_offset=None,
                    in_=tabs,
                    in_offset=bass.IndirectOffsetOnAxis(
                        ap=idx_sb[:, c : c + 1], axis=0
                    ),
                )

        def setup_consts():
            # emitted after the first gather burst so gathers start first
            nc.gpsimd.memset(bias_o[:], -4.0)
            make_identity(nc, ident[:])
            nc.vector.tensor_copy(out=ident8[:], in_=ident[:])
            # preload the exp activation-table set while gathers stream
            warm = P.tile([128, 1], F32, tag="warm")
            act_order(nc.scalar.activation(
                out=warm[:], in_=bias_o[:],
                func=mybir.ActivationFunctionType.Exp,
            ))

        def normalize(c0, c1, tag, cast8):
            nb = c1 - c0
            sq = W.tile([128, nb * DIM], F32, tag="sq", name=f"sq_{tag}")
            g3 = gath[:, c0 * DIM : c1 * DIM].rearrange("p (c d) -> p c d", d=DIM)
            sq_inst = nc.vector.tensor_tensor(out=sq[:], in0=g3, in1=g3, op=op.mult)
            if last_dve[0] is not None:
                add_dep_helper(
                    sq_inst.ins, last_dve[0].ins, sync=False,
                    reason="dve pipeline order",
                )
            nc.vector.tensor_reduce(
                out=nsq[:, c0:c1],
                in_=sq[:].rearrange("p (c d) -> p c d", d=DIM),
                axis=mybir.AxisListType.X,
                op=op.add,
            )
            _emit_rsqrt(nc, W, nsq[:, c0:c1], rinv[:, c0:c1], nb, f"nw_{tag}")
            r3 = (
                rinv[:, c0:c1]
                .rearrange("p (c o) -> p c o", o=1)
                .to_broadcast([128, nb, DIM])
            )
            nc.vector.tensor_tensor(out=g3, in0=g3, in1=r3, op=op.mult)
            if cast8:
                last_dve[0] = nc.vector.tensor_copy(
                    out=gath8[:, c0 * DIM : c1 * DIM],
                    in_=gath[:, c0 * DIM : c1 * DIM],
                )
            else:
                last_dve[0] = None

        def transpose_group(g):
            """8 fp8 transposes (4 bands x 2 halves) -> [32,1024] PSUM, then
            Sync-DMA the packed group into xnT8."""
            # fp8 transpose outputs must land at element step 2 in PSUM, so
            # the tile is double-width and read back at stride 2.
            pt8 = PS.tile([32, 2048], F8, tag="ps", name=f"tp{g}")
            for bi in range(4):
                c = g * 4 + bi
                for h in range(2):
                    s = 2 * (h * 512 + bi * 128)
                    pe_order(nc.tensor.transpose(
                        out=pt8[0:32, s : s + 256 : 2],
                        in_=gath8[:, c * DIM + h * 32 : c * DIM + (h + 1) * 32],
                        identity=ident8[:],
                    ))
            cp = nc.vector.tensor_copy(
                out=xnT8[:, g * 1024 : (g + 1) * 1024], in_=pt8[0:32, 0:2048:2]
            )
            if last_dve[0] is not None:
                add_dep_helper(cp.ins, last_dve[0].ins, sync=False,
                               reason="dve pipeline order")
            last_dve[0] = cp

        def transpose_chunk(ci):
            transpose_group(2 * ci)
            transpose_group(2 * ci + 1)

        def rhs_ap(g, co, w):
            return xnT8[:, g * 1024 : (g + 1) * 1024].rearrange(
                "p (h c) -> p h c", h=2
            )[:, :, co : co + w]

        def lhs_ap(q, rt):
            return rhs_ap(q * 2 + rt // 4, (rt % 4) * 128, 128)

        # ---- rolling off-diagonal emitter: uniform bias, 2048-wide ACTs ----
        st = {"tile": None, "fill": 0, "n": 0}
        colctr = [1, 18]
        dvecol = [35]
        dveflip = [0]
        late = [False]

        def mm_piece(q, rt, g, co, w):
            lhs = lhs_ap(q, rt)
            while w > 0:
                if st["tile"] is None:
                    st["tile"] = PS.tile(
                        [128, PSW], F32, tag="ps", name=f"mm{st['n']}"
                    )
                    st["n"] += 1
                # a matmul output cannot cross a 512-col PSUM bank boundary
                take = min(w, PSW - st["fill"], 512 - st["fill"] % 512)
                pe_order(nc.tensor.matmul(
                    out=st["tile"][:, st["fill"] : st["fill"] + take],
                    lhsT=lhs,
                    rhs=rhs_ap(g, co, take),
                    start=True,
                    stop=True,
                    perf_mode=DR,
                ))
                st["fill"] += take
                co += take
                w -= take
                if st["fill"] == PSW:
                    flush(q)

        def flush(q):
            if st["fill"]:
                dve_ok = late[0]
                use_dve = dve_ok and dveflip[0] % 2 == 0
                if dve_ok:
                    dveflip[0] += 1
                if use_dve:
                    # late-stage tile to the otherwise-idle DVE: bitcast exp
                    # (tensor_scalar affine -> i32, f32-bitcast reduce)
                    col = dvecol[0]
                    dvecol[0] += 1
                    w = st["fill"]
                    cv = W.tile([128, PSW], I32, tag="conv", name=f"cv{col}")
                    ts = nc.vector.tensor_scalar(
                        out=cv[:, 0:w], in0=st["tile"][:, 0:w],
                        scalar1=A_SCH, scalar2=B_SCH,
                        op0=op.mult, op1=op.add,
                    )
                    if last_dve[0] is not None:
                        add_dep_helper(ts.ins, last_dve[0].ins, sync=False,
                                       reason="dve order")
                    tr = nc.vector.tensor_reduce(
                        out=accw[:, col : col + 1],
                        in_=cv[:, 0:w].bitcast(F32),
                        axis=mybir.AxisListType.X,
                        op=op.add,
                    )
                    add_dep_helper(tr.ins, ts.ins, sync=False, reason="dve order")
                    last_dve[0] = tr
                else:
                    col = colctr[q]
                    colctr[q] += 1
                    act_order(nc.scalar.activation(
                        out=st["tile"][:, 0 : st["fill"]],
                        in_=st["tile"][:, 0 : st["fill"]],
                        func=mybir.ActivationFunctionType.Exp,
                        bias=bias_o[:],
                        scale=4.0,
                        accum_out=accw[:, col : col + 1],
                    ))
            st["tile"] = None
            st["fill"] = 0

        def self_stage(q):
            # 8 [128,128] self tiles of diag chunk q -> separate accum column
            t = PS.tile([128, PSW], F32, tag="ps", name=f"self{q}")
            for rt in range(8):
                pe_order(nc.tensor.matmul(
                    out=t[:, rt * 128 : (rt + 1) * 128],
                    lhsT=lhs_ap(q, rt),
                    rhs=lhs_ap(q, rt),
                    start=True,
                    stop=True,
                    perf_mode=DR,
                ))
            col = q * 17
            act_order(nc.scalar.activation(
                out=t[:, 0:1024],
                in_=t[:, 0:1024],
                func=mybir.ActivationFunctionType.Exp,
                bias=bias_o[:],
                scale=4.0,
                accum_out=accw[:, col : col + 1],
            ))

        def up_stage(q):
            # strict upper triangle of diag chunk q at 128-tile granularity
            for rt in range(8):
                s = (rt + 1) * 128
                for lo, hi in ((s, 512), (max(s, 512), 1024)):
                    if hi > lo:
                        mm_piece(q, rt, q * 2 + lo // 512, lo % 512, hi - lo)

        def o1_stage(q, cr):
            # diag chunk q rows vs full chunk cr (8192 cols = 4 ACTs)
            for rt in range(8):
                mm_piece(q, rt, cr * 2, 0, 512)
                mm_piece(q, rt, cr * 2 + 1, 0, 512)
            flush(q)

        def o2_stage(q):
            # distance-4 half block: rows rt vs one 512-row half of chunk q+4
            # (halves swapped for a1>=4 via the host-built gather order)
            for rt in range(8):
                g = (q + 4) * 2 + (0 if rt < 4 else 1)
                mm_piece(q, rt, g, 0, 512)
            flush(q)

        # ---- emission ----
        gather_chunk(0)
        setup_consts()
        for ch in range(1, 6):
            gather_chunk(ch)

        normalize(0, 8, "c0", True)
        transpose_chunk(0)
        self_stage(0)
        up_stage(0)
        normalize(8, 16, "c1", True)
        transpose_chunk(1)
        self_stage(1)
        up_stage(1)
        o1_stage(0, 1)
        normalize(16, 24, "c2", True)
        transpose_chunk(2)
        o1_stage(1, 2)
        o1_stage(0, 2)
        normalize(24, 32, "c3", True)
        transpose_chunk(3)
        o1_stage(1, 3)
        o1_stage(0, 3)
        normalize(32, 40, "c4", True)
        transpose_chunk(4)
        late[0] = True
        o1_stage(1, 4)
        o2_stage(0)
        normalize(40, 48, "c5", True)
        transpose_chunk(5)
        o2_stage(1)

        nc.sync.dma_start(out=acc, in_=accw[:])


def _build():
    nc = bacc.Bacc(
        "TRN2",
        target_bir_lowering=False,
        debug=False,
        enable_asserts=False,
        num_devices=NCORES,
    )
    tabs = nc.dram_tensor("tabs", [2 * NROWS, DIM], F32, kind="ExternalInput").ap()
    gidx = nc.dram_tensor("gidx", [128, NBAND], I32, kind="ExternalInput").ap()
    acc = nc.dram_tensor("acc", [128, ACC_W], F32, kind="ExternalOutput").ap()
    with tile.TileContext(nc) as tc:
        _body(tc, tabs, gidx, acc)
    nc.compile()
    return nc


_PROG = None


def _get_prog():
    global _PROG
    if _PROG is None:
        _PROG = _build()
    return _PROG


def _core_params(m):
    """core m -> (table t, first assignment a1)."""
    t = 0 if m < 4 else 1
    j = m % 4
    a1 = 2 * j + t  # u-cores: 0,2,4,6; p-cores: 1,3,5,7
    return t, a1


def _core_gidx(uid, pid, m):
    """[128, NBAND] int32 gather indices for core m (into the concat table)."""
    t, a1 = _core_params(m)
    main_ids = [uid, pid][t]
    ch = main_ids.reshape(NCORES, CHUNK)

    def h(a):  # quadrant half order for assignment a
        return 0 if a < 4 else 1

    segs = []
    for i in range(NCHUNK):
        cids = ch[(a1 + i) % NCORES].astype(np.int64) + t * NROWS
        if i == 4 and h(a1) == 1:
            cids = np.concatenate([cids[512:], cids[:512]])
        if i == 5 and h((a1 + 1) % NCORES) == 1:
            cids = np.concatenate([cids[512:], cids[:512]])
        segs.append(cids)
    slots = np.concatenate(segs).astype(np.int32)
    assert slots.shape == (NBAND * 128,)
    return np.ascontiguousarray(slots.reshape(NBAND, 128).T)


def _make_in_maps(user_id, pos_id, user_table, item_table):
    tabs = np.ascontiguousarray(
        np.concatenate(
            [
                np.asarray(user_table, dtype=np.float32),
                np.asarray(item_table, dtype=np.float32),
            ],
            axis=0,
        )
    )
    uid = np.asarray(user_id).astype(np.int64)
    pid = np.asarray(pos_id).astype(np.int64)
    return [
        {"tabs": tabs, "gidx": _core_gidx(uid, pid, m)} for m in range(NCORES)
    ]


def _host_align(user_id, pos_id, user_table, item_table):
    """align term in f64 on the host: 0.01% of the FLOPs, but 8 of 56 gather
    bands on the device's pacing GpSimd queue."""
    ue = np.asarray(user_table, dtype=np.float64)[np.asarray(user_id)]
    pe = np.asarray(item_table, dtype=np.float64)[np.asarray(pos_id)]
    un = ue / np.linalg.norm(ue, axis=1, keepdims=True)
    pn = pe / np.linalg.norm(pe, axis=1, keepdims=True)
    return 2.0 - (2.0 / B) * float(np.einsum("ij,ij->", un, pn))


def _finalize(accs, align):
    """accs: list of [128, ACC_W] per core -> scalar loss."""
    a = np.stack([np.asarray(x, dtype=np.float64) for x in accs])
    off_cols = [c for c in range(ACC_W) if c not in SELF_COLS]
    s_off_u = a[0:4][:, :, off_cols].sum()
    s_self_u = a[0:4][:, :, list(SELF_COLS)].sum()
    s_off_p = a[4:8][:, :, off_cols].sum()
    s_self_p = a[4:8][:, :, list(SELF_COLS)].sum()
    npairs = B * (B - 1) // 2
    pair_u = s_off_u + (s_self_u - B) / 2.0
    pair_p = s_off_p + (s_self_p - B) / 2.0
    unif = 0.5 * (np.log(pair_u / npairs) + np.log(pair_p / npairs))
    return np.asarray(align + unif, dtype=np.float32)


def _run(in_maps, trace=False, **kw):
    nc = _get_prog()
    return bass_utils.run_bass_kernel_spmd(
        nc, in_maps, core_ids=list(range(NCORES)), trace=trace, **kw
    )


def kernel(user_id, pos_id, neg_id=None, user_table=None, item_table=None):
    in_maps = _make_in_maps(user_id, pos_id, user_table, item_table)
    align = _host_align(user_id, pos_id, user_table, item_table)
    res = _run(in_maps, trace=False)
    return _finalize([res.results[m]["acc"] for m in range(NCORES)], align)


def _install_profile_hook():
    """The image's antenv lacks axon_hooks; shim it so trace=True can reach
    the NTFF profiler in libaxon_pjrt.so (same mechanism trn_boot uses)."""
    import sys
    import types

    if "antenv.axon_hooks" in sys.modules:
        return
    import antenv
    from trn_agent_boot.trn_boot import _ntff_profile_via_ctypes

    mod = types.ModuleType("antenv.axon_hooks")
    holder = [None]
    mod.set_axon_ntff_profile_hook = lambda h: holder.__setitem__(0, h)
    mod.get_axon_ntff_profile_hook = lambda: holder[0]
    sys.modules["antenv.axon_hooks"] = mod
    antenv.axon_hooks = mod
    mod.set_axon_ntff_profile_hook(
        _ntff_profile_via_ctypes("/opt/axon/libaxon_pjrt.so")
    )
    # no bucket filesystem in this container
    bass_utils.upload_artifacts = lambda tmpdir: ""


def run_profiled(user_id, pos_id, neg_id=None, user_table=None, item_table=None, **kw):
    _install_profile_hook()
    in_maps = _make_in_maps(user_id, pos_id, user_table, item_table)
    align = _host_align(user_id, pos_id, user_table, item_table)
    res = _run(in_maps, trace=True, **kw)
    out = _finalize([res.results[m]["acc"] for m in range(NCORES)], align)
    return out, res


# revision 43
# speedup vs baseline: 1.0683x; 1.0683x over previous
"""DirectAU loss kernel for Trainium2, SPMD over 8 NeuronCores.

Math (see reference):
  user_e = user_table[user_id]; pos_e = item_table[pos_id]   (B=8192, D=64)
  align  = mean_i ||un_i - pn_i||^2 = 2 - (2/B) sum_i <un_i, pn_i>
  unif(x)= log( (sum_{i<j} exp(-4 + 4 <xn_i, xn_j>)) / npairs )
  out    = align + 0.5*(unif(user_e) + unif(pos_e))

Strategy (v4 pipeline + host-side align):
  - Work split: cores 0-3 the user-table Gram, 4-7 the pos-table
    one; both tables concatenated so the SPMD program is identical and the
    choice lives in the int32 gather indices. Each core owns two adjacent
    1024-row chunk assignments {a1, a1+1} of its table and gathers chunks
    a1..a1+5.
  - The align term is 0.01% of the FLOPs but costs 8 of the 56 gather
    bands (~9us of SWDGE issue on the pacing GpSimd queue); it is folded
    into the host finalization (which already applies the closed-form
    log / diagonal corrections) in float64.
  - Gram matmuls run in fp8-e4m3 with MatmulPerfMode.DoubleRow (0.5
    cycles/row, 2x bf16): normalized rows are cast to fp8 (DVE), PE-
    transposed per 32-dim half into [32, 2, 512]-packed k-tile layout
    (dims 0-31 in k-tile 0, 32-63 in k-tile 1), and copied PSUM->SBUF.
  - Exp bias is uniform (-4) for every tile: diagonal chunks are computed
    strict-upper-triangular at 128-tile granularity, and the 8 self tiles
    [rt,rt] of each diag chunk go to a separate accumulator column; the
    host removes the double-count closed-form ((S_self - B)/2). This packs
    all off-diagonal exp work into 2048-wide ACTIVATEs regardless of block
    structure, minimizing the ~520ns/instr ACT overhead.
  - ACT (scalar engine) exp at 0.833ns/col is the pacer; emission orders
    PE/ACT/DVE queues explicitly (order-only deps) so the in-order engines
    never reorder into stall-prone sequences.
  - Host sums the 8x[128,35] partials and applies the closed-form
    log/align finalization.
"""

import math

import numpy as np

import concourse.bacc as bacc
import concourse.bass as bass
import concourse.mybir as mybir
import concourse.tile as tile
from concourse import bass_utils
from concourse.masks import make_identity
from concourse.tile_rust import add_dep_helper

B = 8192
DIM = 64
NROWS = 100000
NCORES = 8
CHUNK = 1024
NCHUNK = 6  # gathered main chunks per core (C0..C5)
MAIN_BANDS = NCHUNK * 8  # 48
NBAND = MAIN_BANDS  # 48 gather bands (align is host-side)
NGRP = NCHUNK * 2  # 12 transpose groups of 4 bands (512 rows)
F32 = mybir.dt.float32
F8 = mybir.dt.float8e4
I32 = mybir.dt.int32

# accumulator columns: q in {0,1}: col q*17 = self-tile sums of diag chunk q,
# cols q*17+1 .. q*17+16 = off-diagonal exp sums
SELF_COLS = (0, 17)
ACC_W = 42
PSW = 2048  # PSUM work tile width (fp32)

# Schraudolph fast-exp for exp(4s-4): i32 = s*A + B_, bitcast f32. B_ is
# calibrated (C=-480000) to ~2e-4 mean bias over <xn_i,xn_j> ~ N(0,1/64);
# used only for late drain tiles handed to the otherwise-idle DVE.
A_SCH = float(np.float32(4.0 * (2.0 ** 23) / math.log(2.0)))
B_SCH = float(np.float32(127 * 2 ** 23 - 4.0 * (2.0 ** 23) / math.log(2.0) - 480000.0))


def _emit_rsqrt(nc, pool, x_ap, out_ap, n, tag):
    """out = 1/sqrt(x) on the vector engine (bit-hack seed + 3 Newton steps)."""
    MAGIC = 0x5F3759DF
    op = mybir.AluOpType
    ti = pool.tile([128, n], I32, tag=f"{tag}_ti", name=f"{tag}_ti")
    nc.vector.tensor_scalar(
        out=ti[:], in0=x_ap.bitcast(I32), scalar1=1, scalar2=None,
        op0=op.logical_shift_right,
    )
    yi = pool.tile([128, n], I32, tag=f"{tag}_yi", name=f"{tag}_yi")
    # MAGIC - t == (t ^ -1) + (MAGIC + 1); split: ISA can't mix bitwise+arith
    nc.vector.tensor_scalar(
        out=yi[:], in0=ti[:], scalar1=-1, scalar2=None, op0=op.bitwise_xor
    )
    nc.vector.tensor_scalar(
        out=yi[:], in0=yi[:], scalar1=MAGIC + 1, scalar2=None, op0=op.add
    )
    xh = pool.tile([128, n], F32, tag=f"{tag}_xh", name=f"{tag}_xh")
    nc.vector.tensor_scalar(
        out=xh[:], in0=x_ap, scalar1=-0.5, scalar2=None, op0=op.mult
    )
    cur = yi[:].bitcast(F32)
    for it in range(3):
        t2 = pool.tile([128, n], F32, tag=f"{tag}_t2", name=f"{tag}_t2")
        nc.vector.tensor_mul(out=t2[:], in0=cur, in1=cur)
        nc.vector.tensor_mul(out=t2[:], in0=t2[:], in1=xh[:])
        nc.vector.tensor_scalar(
            out=t2[:], in0=t2[:], scalar1=1.5, scalar2=None, op0=op.add
        )
        if it == 2:
            dst_ap = out_ap
        else:
            yt = pool.tile([128, n], F32, tag=f"{tag}_y", name=f"{tag}_y{it}")
            dst_ap = yt[:]
        nc.vector.tensor_mul(out=dst_ap, in0=cur, in1=t2[:])
        cur = dst_ap
    return cur


def _body(tc, tabs, gidx, acc):
    nc = tc.nc
    op = mybir.AluOpType
    DR = mybir.MatmulPerfMode.DoubleRow
    with (
        tc.tile_pool(name="persist", bufs=1) as P,
        tc.tile_pool(name="work", bufs=2) as W,
        tc.tile_pool(name="ps", bufs=2, space="PSUM") as PS,
    ):
        idx_sb = P.tile([128, NBAND], I32, tag="idx")
        nc.sync.dma_start(out=idx_sb[:], in_=gidx)

        accw = P.tile([128, ACC_W], F32, tag="accw")
        nc.vector.memset(accw[:], 0.0)
        bias_o = P.tile([128, 1], F32, tag="bias_o")
        ident = P.tile([128, 128], F32, tag="ident")
        ident8 = P.tile([128, 128], F8, tag="ident8")

        # gathered rows, [128, band, DIM] band-major slots (row c*128+p)
        gath = P.tile([128, NBAND * DIM], F32, tag="gath")
        gath8 = P.tile([128, MAIN_BANDS * DIM], F8, tag="gath8")
        # fp8 transposed layout: group g (4 bands = 512 rows) occupies cols
        # [g*1024, (g+1)*1024): [32 partitions, k-half h in {0,1}, 512 rows];
        # dim d of row r lives at partition d%32, half d//32, col r.
        xnT8 = P.tile([32, NGRP * 1024], F8, tag="xnT8")
        nsq = P.tile([128, NBAND], F32, tag="nsq")
        rinv = P.tile([128, NBAND], F32, tag="rinv")

        # Queue-order pinning: the scheduler's cost model mis-predicts gather
        # and PE readiness, which otherwise reorders the in-order engine
        # queues into stall-prone sequences. Chain DVE normalize stages, and
        # pin the PE and ACT queues to emission order with order-only deps.
        last_dve = [None]
        last_pe = [None]
        last_act = [None]

        def pe_order(inst):
            if last_pe[0] is not None:
                add_dep_helper(inst.ins, last_pe[0].ins, sync=False,
                               reason="pe order")
            last_pe[0] = inst

        def act_order(inst):
            if last_act[0] is not None:
                add_dep_helper(inst.ins, last_act[0].ins, sync=False,
                               reason="act order")
            last_act[0] = inst

        def gather_chunk(ch):
            # NOTE: one indirect DMA per 128-row band. Batching several offset
            # columns into one issue generates corrupt descriptors on HW
            # (wrong rows, byte-misaligned payloads) — do not batch.
            for c in range(ch * 8, (ch + 1) * 8):
                nc.gpsimd.indirect_dma_start(
                    out=gath[:, c * DIM : (c + 1) * DIM],
                    out_offset=None,
                    in_=tabs,
                    in_offset=bass.IndirectOffsetOnAxis(
                        ap=idx_sb[:, c : c + 1], axis=0
                    ),
                )

        def setup_consts():
            # emitted after the first gather burst so gathers start first
            nc.gpsimd.memset(bias_o[:], -4.0)
            make_identity(nc, ident[:])
            nc.vector.tensor_copy(out=ident8[:], in_=ident[:])
            # preload the exp activation-table set while gathers stream
            warm = P.tile([128, 1], F32, tag="warm")
            act_order(nc.scalar.activation(
                out=warm[:], in_=bias_o[:],
                func=mybir.ActivationFunctionType.Exp,
            ))

        def normalize(c0, c1, tag, cast8):
            nb = c1 - c0
            sq = W.tile([128, nb * DIM], F32, tag="sq", name=f"sq_{tag}")
            g3 = gath[:, c0 * DIM : c1 * DIM].rearrange("p (c d) -> p c d", d=DIM)
            sq_inst = nc.vector.tensor_tensor(out=sq[:], in0=g3, in1=g3, op=op.mult)
            if last_dve[0] is not None:
                add_dep_helper(
                    sq_inst.ins, last_dve[0].ins, sync=False,
                    reason="dve pipeline order",
                )
            nc.vector.tensor_reduce(
                out=nsq[:, c0:c1],
                in_=sq[:].rearrange("p (c d) -> p c d", d=DIM),
                axis=mybir.AxisListType.X,
                op=op.add,
            )
            _emit_rsqrt(nc, W, nsq[:, c0:c1], rinv[:, c0:c1], nb, f"nw_{tag}")
            r3 = (
                rinv[:, c0:c1]
                .rearrange("p (c o) -> p c o", o=1)
                .to_broadcast([128, nb, DIM])
            )
            nc.vector.tensor_tensor(out=g3, in0=g3, in1=r3, op=op.mult)
            if cast8:
                last_dve[0] = nc.vector.tensor_copy(
                    out=gath8[:, c0 * DIM : c1 * DIM],
                    in_=gath[:, c0 * DIM : c1 * DIM],
                )
            else:
                last_dve[0] = None

        def transpose_group(g):
            """8 fp8 transposes (4 bands x 2 halves) -> [32,1024] PSUM, then
            Sync-DMA the packed group into xnT8."""
            # fp8 transpose outputs must land at element step 2 in PSUM, so
            # the tile is double-width and read back at stride 2.
            pt8 = PS.tile([32, 2048], F8, tag="ps", name=f"tp{g}")
            for bi in range(4):
                c = g * 4 + bi
                for h in range(2):
                    s = 2 * (h * 512 + bi * 128)
                    pe_order(nc.tensor.transpose(
                        out=pt8[0:32, s : s + 256 : 2],
                        in_=gath8[:, c * DIM + h * 32 : c * DIM + (h + 1) * 32],
                        identity=ident8[:],
                    ))
            cp = nc.vector.tensor_copy(
                out=xnT8[:, g * 1024 : (g + 1) * 1024], in_=pt8[0:32, 0:2048:2]
            )
            if last_dve[0] is not None:
                add_dep_helper(cp.ins, last_dve[0].ins, sync=False,
                               reason="dve pipeline order")
            last_dve[0] = cp

        def transpose_chunk(ci):
            transpose_group(2 * ci)
            transpose_group(2 * ci + 1)

        def rhs_ap(g, co, w):
            return xnT8[:, g * 1024 : (g + 1) * 1024].rearrange(
                "p (h c) -> p h c", h=2
            )[:, :, co : co + w]

        def lhs_ap(q, rt):
            return rhs_ap(q * 2 + rt // 4, (rt % 4) * 128, 128)

        # ---- rolling off-diagonal emitter: uniform bias, 2048-wide ACTs ----
        st = {"tile": None, "fill": 0, "n": 0}
        colctr = [1, 18]
        dvecol = [35]
        dveflip = [0]
        late = [False]

        def mm_piece(q, rt, g, co, w):
            lhs = lhs_ap(q, rt)
            while w > 0:
                if st["tile"] is None:
                    st["tile"] = PS.tile(
                        [128, PSW], F32, tag="ps", name=f"mm{st['n']}"
                    )
                    st["n"] += 1
                # a matmul output cannot cross a 512-col PSUM bank boundary
                take = min(w, PSW - st["fill"], 512 - st["fill"] % 512)
                pe_order(nc.tensor.matmul(
                    out=st["tile"][:, st["fill"] : st["fill"] + take],
                    lhsT=lhs,
                    rhs=rhs_ap(g, co, take),
                    start=True,
                    stop=True,
                    perf_mode=DR,
                ))
                st["fill"] += take
                co += take
                w -= take
                if st["fill"] == PSW:
                    flush(q)

        def flush(q):
            if st["fill"]:
                dve_ok = late[0]
                use_dve = dve_ok and dveflip[0] % 2 == 0
                if dve_ok:
                    dveflip[0] += 1
                if use_dve:
                    # late-stage tile to the otherwise-idle DVE: bitcast exp
                    # (tensor_scalar affine -> i32, f32-bitcast reduce)
                    col = dvecol[0]
                    dvecol[0] += 1
                    w = st["fill"]
                    cv = W.tile([128, PSW], I32, tag="conv", name=f"cv{col}")
                    ts = nc.vector.tensor_scalar(
                        out=cv[:, 0:w], in0=st["tile"][:, 0:w],
                        scalar1=A_SCH, scalar2=B_SCH,
                        op0=op.mult, op1=op.add,
                    )
                    if last_dve[0] is not None:
                        add_dep_helper(ts.ins, last_dve[0].ins, sync=False,
                                       reason="dve order")
                    tr = nc.vector.tensor_reduce(
                        out=accw[:, col : col + 1],
                        in_=cv[:, 0:w].bitcast(F32),
                        axis=mybir.AxisListType.X,
                        op=op.add,
                    )
                    add_dep_helper(tr.ins, ts.ins, sync=False, reason="dve order")
                    last_dve[0] = tr
                else:
                    col = colctr[q]
                    colctr[q] += 1
                    act_order(nc.scalar.activation(
                        out=st["tile"][:, 0 : st["fill"]],
                        in_=st["tile"][:, 0 : st["fill"]],
                        func=mybir.ActivationFunctionType.Exp,
                        bias=bias_o[:],
                        scale=4.0,
                        accum_out=accw[:, col : col + 1],
                    ))
            st["tile"] = None
            st["fill"] = 0

        def self_stage(q):
            # 8 [128,128] self tiles of diag chunk q -> separate accum column
            t = PS.tile([128, PSW], F32, tag="ps", name=f"self{q}")
            for rt in range(8):
                pe_order(nc.tensor.matmul(
                    out=t[:, rt * 128 : (rt + 1) * 128],
                    lhsT=lhs_ap(q, rt),
                    rhs=lhs_ap(q, rt),
                    start=True,
                    stop=True,
                    perf_mode=DR,
                ))
            col = q * 17
            act_order(nc.scalar.activation(
                out=t[:, 0:1024],
                in_=t[:, 0:1024],
                func=mybir.ActivationFunctionType.Exp,
                bias=bias_o[:],
                scale=4.0,
                accum_out=accw[:, col : col + 1],
            ))

        def up_stage(q):
            # strict upper triangle of diag chunk q at 128-tile granularity
            for rt in range(8):
                s = (rt + 1) * 128
                for lo, hi in ((s, 512), (max(s, 512), 1024)):
                    if hi > lo:
                        mm_piece(q, rt, q * 2 + lo // 512, lo % 512, hi - lo)

        def o1_stage(q, cr):
            # diag chunk q rows vs full chunk cr (8192 cols = 4 ACTs)
            for rt in range(8):
                mm_piece(q, rt, cr * 2, 0, 512)
                mm_piece(q, rt, cr * 2 + 1, 0, 512)
            flush(q)

        def o2_stage(q):
            # distance-4 half block: rows rt vs one 512-row half of chunk q+4
            # (halves swapped for a1>=4 via the host-built gather order)
            for rt in range(8):
                g = (q + 4) * 2 + (0 if rt < 4 else 1)
                mm_piece(q, rt, g, 0, 512)
            flush(q)

        # ---- emission ----
        gather_chunk(0)
        setup_consts()
        for ch in range(1, 6):
            gather_chunk(ch)

        normalize(0, 8, "c0", True)
        transpose_chunk(0)
        self_stage(0)
        up_stage(0)
        normalize(8, 16, "c1", True)
        transpose_chunk(1)
        self_stage(1)
        up_stage(1)
        o1_stage(0, 1)
        normalize(16, 24, "c2", True)
        transpose_chunk(2)
        o1_stage(1, 2)
        o1_stage(0, 2)
        normalize(24, 32, "c3", True)
        transpose_chunk(3)
        o1_stage(1, 3)
        o1_stage(0, 3)
        normalize(32, 40, "c4", True)
        transpose_chunk(4)
        o1_stage(1, 4)
        o2_stage(0)
        normalize(40, 48, "c5", True)
        transpose_chunk(5)
        late[0] = True
        o2_stage(1)

        nc.sync.dma_start(out=acc, in_=accw[:])


def _build():
    nc = bacc.Bacc(
        "TRN2",
        target_bir_lowering=False,
        debug=False,
        enable_asserts=False,
        num_devices=NCORES,
    )
    tabs = nc.dram_tensor("tabs", [2 * NROWS, DIM], F32, kind="ExternalInput").ap()
    gidx = nc.dram_tensor("gidx", [128, NBAND], I32, kind="ExternalInput").ap()
    acc = nc.dram_tensor("acc", [128, ACC_W], F32, kind="ExternalOutput").ap()
    with tile.TileContext(nc) as tc:
        _body(tc, tabs, gidx, acc)
    nc.compile()
    return nc


_PROG = None


def _get_prog():
    global _PROG
    if _PROG is None:
        _PROG = _build()
    return _PROG


def _core_params(m):
    """core m -> (table t, first assignment a1)."""
    t = 0 if m < 4 else 1
    j = m % 4
    a1 = 2 * j + t  # u-cores: 0,2,4,6; p-cores: 1,3,5,7
    return t, a1


def _core_gidx(uid, pid, m):
    """[128, NBAND] int32 gather indices for core m (into the concat table)."""
    t, a1 = _core_params(m)
    main_ids = [uid, pid][t]
    ch = main_ids.reshape(NCORES, CHUNK)

    def h(a):  # quadrant half order for assignment a
        return 0 if a < 4 else 1

    segs = []
    for i in range(NCHUNK):
        cids = ch[(a1 + i) % NCORES].astype(np.int64) + t * NROWS
        if i == 4 and h(a1) == 1:
            cids = np.concatenate([cids[512:], cids[:512]])
        if i == 5 and h((a1 + 1) % NCORES) == 1:
            cids = np.concatenate([cids[512:], cids[:512]])
        segs.append(cids)
    slots = np.concatenate(segs).astype(np.int32)
    assert slots.shape == (NBAND * 128,)
    return np.ascontiguousarray(slots.reshape(NBAND, 128).T)


def _make_in_maps(user_id, pos_id, user_table, item_table):
    tabs = np.ascontiguousarray(
        np.concatenate(
            [
                np.asarray(user_table, dtype=np.float32),
                np.asarray(item_table, dtype=np.float32),
            ],
            axis=0,
        )
    )
    uid = np.asarray(user_id).astype(np.int64)
    pid = np.asarray(pos_id).astype(np.int64)
    return [
        {"tabs": tabs, "gidx": _core_gidx(uid, pid, m)} for m in range(NCORES)
    ]


def _host_align(user_id, pos_id, user_table, item_table):
    """align term in f64 on the host: 0.01% of the FLOPs, but 8 of 56 gather
    bands on the device's pacing GpSimd queue."""
    ue = np.asarray(user_table, dtype=np.float64)[np.asarray(user_id)]
    pe = np.asarray(item_table, dtype=np.float64)[np.asarray(pos_id)]
    un = ue / np.linalg.norm(ue, axis=1, keepdims=True)
    pn = pe / np.linalg.norm(pe, axis=1, keepdims=True)
    return 2.0 - (2.0 / B) * float(np.einsum("ij,ij->", un, pn))


def _finalize(accs, align):
    """accs: list of [128, ACC_W] per core -> scalar loss."""
    a = np.stack([np.asarray(x, dtype=np.float64) for x in accs])
    off_cols = [c for c in range(ACC_W) if c not in SELF_COLS]
    s_off_u = a[0:4][:, :, off_cols].sum()
    s_self_u = a[0:4][:, :, list(SELF_COLS)].sum()
    s_off_p = a[4:8][:, :, off_cols].sum()
    s_self_p = a[4:8][:, :, list(SELF_COLS)].sum()
    npairs = B * (B - 1) // 2
    pair_u = s_off_u + (s_self_u - B) / 2.0
    pair_p = s_off_p + (s_self_p - B) / 2.0
    unif = 0.5 * (np.log(pair_u / npairs) + np.log(pair_p / npairs))
    return np.asarray(align + unif, dtype=np.float32)


def _run(in_maps, trace=False, **kw):
    nc = _get_prog()
    return bass_utils.run_bass_kernel_spmd(
        nc, in_maps, core_ids=list(range(NCORES)), trace=trace, **kw
    )


def kernel(user_id, pos_id, neg_id=None, user_table=None, item_table=None):
    in_maps = _make_in_maps(user_id, pos_id, user_table, item_table)
    align = _host_align(user_id, pos_id, user_table, item_table)
    res = _run(in_maps, trace=False)
    return _finalize([res.results[m]["acc"] for m in range(NCORES)], align)


def _install_profile_hook():
    """The image's antenv lacks axon_hooks; shim it so trace=True can reach
    the NTFF profiler in libaxon_pjrt.so (same mechanism trn_boot uses)."""
    import sys
    import types

    if "antenv.axon_hooks" in sys.modules:
        return
    import antenv
    from trn_agent_boot.trn_boot import _ntff_profile_via_ctypes

    mod = types.ModuleType("antenv.axon_hooks")
    holder = [None]
    mod.set_axon_ntff_profile_hook = lambda h: holder.__setitem__(0, h)
    mod.get_axon_ntff_profile_hook = lambda: holder[0]
    sys.modules["antenv.axon_hooks"] = mod
    antenv.axon_hooks = mod
    mod.set_axon_ntff_profile_hook(
        _ntff_profile_via_ctypes("/opt/axon/libaxon_pjrt.so")
    )
    # no bucket filesystem in this container
    bass_utils.upload_artifacts = lambda tmpdir: ""


def run_profiled(user_id, pos_id, neg_id=None, user_table=None, item_table=None, **kw):
    _install_profile_hook()
    in_maps = _make_in_maps(user_id, pos_id, user_table, item_table)
    align = _host_align(user_id, pos_id, user_table, item_table)
    res = _run(in_maps, trace=True, **kw)
    out = _finalize([res.results[m]["acc"] for m in range(NCORES)], align)
    return out, res


# revision 44
# speedup vs baseline: 1.0717x; 1.0031x over previous
"""DirectAU loss kernel for Trainium2, SPMD over 8 NeuronCores.

Math (see reference):
  user_e = user_table[user_id]; pos_e = item_table[pos_id]   (B=8192, D=64)
  align  = mean_i ||un_i - pn_i||^2 = 2 - (2/B) sum_i <un_i, pn_i>
  unif(x)= log( (sum_{i<j} exp(-4 + 4 <xn_i, xn_j>)) / npairs )
  out    = align + 0.5*(unif(user_e) + unif(pos_e))

Strategy (v4 pipeline + host-side align):
  - Work split: cores 0-3 the user-table Gram, 4-7 the pos-table
    one; both tables concatenated so the SPMD program is identical and the
    choice lives in the int32 gather indices. Each core owns two adjacent
    1024-row chunk assignments {a1, a1+1} of its table and gathers chunks
    a1..a1+5.
  - The align term is 0.01% of the FLOPs but costs 8 of the 56 gather
    bands (~9us of SWDGE issue on the pacing GpSimd queue); it is folded
    into the host finalization (which already applies the closed-form
    log / diagonal corrections) in float64.
  - Gram matmuls run in fp8-e4m3 with MatmulPerfMode.DoubleRow (0.5
    cycles/row, 2x bf16): normalized rows are cast to fp8 (DVE), PE-
    transposed per 32-dim half into [32, 2, 512]-packed k-tile layout
    (dims 0-31 in k-tile 0, 32-63 in k-tile 1), and copied PSUM->SBUF.
  - Exp bias is uniform (-4) for every tile: diagonal chunks are computed
    strict-upper-triangular at 128-tile granularity, and the 8 self tiles
    [rt,rt] of each diag chunk go to a separate accumulator column; the
    host removes the double-count closed-form ((S_self - B)/2). This packs
    all off-diagonal exp work into 2048-wide ACTIVATEs regardless of block
    structure, minimizing the ~520ns/instr ACT overhead.
  - ACT (scalar engine) exp at 0.833ns/col is the pacer; emission orders
    PE/ACT/DVE queues explicitly (order-only deps) so the in-order engines
    never reorder into stall-prone sequences.
  - Host sums the 8x[128,35] partials and applies the closed-form
    log/align finalization.
"""

import math

import numpy as np

import concourse.bacc as bacc
import concourse.bass as bass
import concourse.mybir as mybir
import concourse.tile as tile
from concourse import bass_utils
from concourse.masks import make_identity
from concourse.tile_rust import add_dep_helper

B = 8192
DIM = 64
NROWS = 100000
NCORES = 8
CHUNK = 1024
NCHUNK = 6  # gathered main chunks per core (C0..C5)
MAIN_BANDS = NCHUNK * 8  # 48
NBAND = MAIN_BANDS  # 48 gather bands (align is host-side)
NGRP = NCHUNK * 2  # 12 transpose groups of 4 bands (512 rows)
F32 = mybir.dt.float32
F8 = mybir.dt.float8e4
I32 = mybir.dt.int32

# accumulator columns: q in {0,1}: col q*17 = self-tile sums of diag chunk q,
# cols q*17+1 .. q*17+16 = off-diagonal exp sums
SELF_COLS = (0, 17)
ACC_W = 42
PSW = 2048  # PSUM work tile width (fp32)

# Schraudolph fast-exp for exp(4s-4): i32 = s*A + B_, bitcast f32. B_ is
# calibrated (C=-480000) to ~2e-4 mean bias over <xn_i,xn_j> ~ N(0,1/64);
# used only for late drain tiles handed to the otherwise-idle DVE.
A_SCH = float(np.float32(4.0 * (2.0 ** 23) / math.log(2.0)))
B_SCH = float(np.float32(127 * 2 ** 23 - 4.0 * (2.0 ** 23) / math.log(2.0) - 480000.0))


def _emit_rsqrt(nc, pool, x_ap, out_ap, n, tag):
    """out = 1/sqrt(x) on the vector engine (bit-hack seed + 3 Newton steps)."""
    MAGIC = 0x5F3759DF
    op = mybir.AluOpType
    ti = pool.tile([128, n], I32, tag=f"{tag}_ti", name=f"{tag}_ti")
    nc.vector.tensor_scalar(
        out=ti[:], in0=x_ap.bitcast(I32), scalar1=1, scalar2=None,
        op0=op.logical_shift_right,
    )
    yi = pool.tile([128, n], I32, tag=f"{tag}_yi", name=f"{tag}_yi")
    # MAGIC - t == (t ^ -1) + (MAGIC + 1); split: ISA can't mix bitwise+arith
    nc.vector.tensor_scalar(
        out=yi[:], in0=ti[:], scalar1=-1, scalar2=None, op0=op.bitwise_xor
    )
    nc.vector.tensor_scalar(
        out=yi[:], in0=yi[:], scalar1=MAGIC + 1, scalar2=None, op0=op.add
    )
    xh = pool.tile([128, n], F32, tag=f"{tag}_xh", name=f"{tag}_xh")
    nc.vector.tensor_scalar(
        out=xh[:], in0=x_ap, scalar1=-0.5, scalar2=None, op0=op.mult
    )
    cur = yi[:].bitcast(F32)
    for it in range(3):
        t2 = pool.tile([128, n], F32, tag=f"{tag}_t2", name=f"{tag}_t2")
        nc.vector.tensor_mul(out=t2[:], in0=cur, in1=cur)
        nc.vector.tensor_mul(out=t2[:], in0=t2[:], in1=xh[:])
        nc.vector.tensor_scalar(
            out=t2[:], in0=t2[:], scalar1=1.5, scalar2=None, op0=op.add
        )
        if it == 2:
            dst_ap = out_ap
        else:
            yt = pool.tile([128, n], F32, tag=f"{tag}_y", name=f"{tag}_y{it}")
            dst_ap = yt[:]
        nc.vector.tensor_mul(out=dst_ap, in0=cur, in1=t2[:])
        cur = dst_ap
    return cur


def _body(tc, tabs, gidx, acc):
    nc = tc.nc
    op = mybir.AluOpType
    DR = mybir.MatmulPerfMode.DoubleRow
    with (
        tc.tile_pool(name="persist", bufs=1) as P,
        tc.tile_pool(name="work", bufs=2) as W,
        tc.tile_pool(name="ps", bufs=2, space="PSUM") as PS,
    ):
        idx_sb = P.tile([128, NBAND], I32, tag="idx")
        nc.sync.dma_start(out=idx_sb[:], in_=gidx)

        accw = P.tile([128, ACC_W], F32, tag="accw")
        nc.vector.memset(accw[:], 0.0)
        bias_o = P.tile([128, 1], F32, tag="bias_o")
        ident = P.tile([128, 128], F32, tag="ident")
        ident8 = P.tile([128, 128], F8, tag="ident8")

        # gathered rows, [128, band, DIM] band-major slots (row c*128+p)
        gath = P.tile([128, NBAND * DIM], F32, tag="gath")
        gath8 = P.tile([128, MAIN_BANDS * DIM], F8, tag="gath8")
        # fp8 transposed layout: group g (4 bands = 512 rows) occupies cols
        # [g*1024, (g+1)*1024): [32 partitions, k-half h in {0,1}, 512 rows];
        # dim d of row r lives at partition d%32, half d//32, col r.
        xnT8 = P.tile([32, NGRP * 1024], F8, tag="xnT8")
        nsq = P.tile([128, NBAND], F32, tag="nsq")
        rinv = P.tile([128, NBAND], F32, tag="rinv")

        # Queue-order pinning: the scheduler's cost model mis-predicts gather
        # and PE readiness, which otherwise reorders the in-order engine
        # queues into stall-prone sequences. Chain DVE normalize stages, and
        # pin the PE and ACT queues to emission order with order-only deps.
        last_dve = [None]
        last_pe = [None]
        last_act = [None]

        def pe_order(inst):
            if last_pe[0] is not None:
                add_dep_helper(inst.ins, last_pe[0].ins, sync=False,
                               reason="pe order")
            last_pe[0] = inst

        def act_order(inst):
            if last_act[0] is not None:
                add_dep_helper(inst.ins, last_act[0].ins, sync=False,
                               reason="act order")
            last_act[0] = inst

        def gather_chunk(ch):
            # NOTE: one indirect DMA per 128-row band. Batching several offset
            # columns into one issue generates corrupt descriptors on HW
            # (wrong rows, byte-misaligned payloads) — do not batch.
            for c in range(ch * 8, (ch + 1) * 8):
                nc.gpsimd.indirect_dma_start(
                    out=gath[:, c * DIM : (c + 1) * DIM],
                    out_offset=None,
                    in_=tabs,
                    in_offset=bass.IndirectOffsetOnAxis(
                        ap=idx_sb[:, c : c + 1], axis=0
                    ),
                )

        def setup_consts():
            # emitted after the first gather burst so gathers start first
            nc.gpsimd.memset(bias_o[:], -4.0)
            make_identity(nc, ident[:])
            nc.vector.tensor_copy(out=ident8[:], in_=ident[:])
            # preload the exp activation-table set while gathers stream
            warm = P.tile([128, 1], F32, tag="warm")
            act_order(nc.scalar.activation(
                out=warm[:], in_=bias_o[:],
                func=mybir.ActivationFunctionType.Exp,
            ))

        def normalize(c0, c1, tag, cast8):
            nb = c1 - c0
            sq = W.tile([128, nb * DIM], F32, tag="sq", name=f"sq_{tag}")
            g3 = gath[:, c0 * DIM : c1 * DIM].rearrange("p (c d) -> p c d", d=DIM)
            sq_inst = nc.vector.tensor_tensor(out=sq[:], in0=g3, in1=g3, op=op.mult)
            if last_dve[0] is not None:
                add_dep_helper(
                    sq_inst.ins, last_dve[0].ins, sync=False,
                    reason="dve pipeline order",
                )
            nc.vector.tensor_reduce(
                out=nsq[:, c0:c1],
                in_=sq[:].rearrange("p (c d) -> p c d", d=DIM),
                axis=mybir.AxisListType.X,
                op=op.add,
            )
            _emit_rsqrt(nc, W, nsq[:, c0:c1], rinv[:, c0:c1], nb, f"nw_{tag}")
            r3 = (
                rinv[:, c0:c1]
                .rearrange("p (c o) -> p c o", o=1)
                .to_broadcast([128, nb, DIM])
            )
            nc.vector.tensor_tensor(out=g3, in0=g3, in1=r3, op=op.mult)
            if cast8:
                last_dve[0] = nc.vector.tensor_copy(
                    out=gath8[:, c0 * DIM : c1 * DIM],
                    in_=gath[:, c0 * DIM : c1 * DIM],
                )
            else:
                last_dve[0] = None

        def transpose_group(g):
            """8 fp8 transposes (4 bands x 2 halves) -> [32,1024] PSUM, then
            Sync-DMA the packed group into xnT8."""
            # fp8 transpose outputs must land at element step 2 in PSUM, so
            # the tile is double-width and read back at stride 2.
            pt8 = PS.tile([32, 2048], F8, tag="ps", name=f"tp{g}")
            for bi in range(4):
                c = g * 4 + bi
                for h in range(2):
                    s = 2 * (h * 512 + bi * 128)
                    pe_order(nc.tensor.transpose(
                        out=pt8[0:32, s : s + 256 : 2],
                        in_=gath8[:, c * DIM + h * 32 : c * DIM + (h + 1) * 32],
                        identity=ident8[:],
                    ))
            cp = nc.vector.tensor_copy(
                out=xnT8[:, g * 1024 : (g + 1) * 1024], in_=pt8[0:32, 0:2048:2]
            )
            if last_dve[0] is not None:
                add_dep_helper(cp.ins, last_dve[0].ins, sync=False,
                               reason="dve pipeline order")
            last_dve[0] = cp

        def transpose_chunk(ci):
            transpose_group(2 * ci)
            transpose_group(2 * ci + 1)

        def rhs_ap(g, co, w):
            return xnT8[:, g * 1024 : (g + 1) * 1024].rearrange(
                "p (h c) -> p h c", h=2
            )[:, :, co : co + w]

        def lhs_ap(q, rt):
            return rhs_ap(q * 2 + rt // 4, (rt % 4) * 128, 128)

        # ---- rolling off-diagonal emitter: uniform bias, 2048-wide ACTs ----
        st = {"tile": None, "fill": 0, "n": 0}
        colctr = [1, 18]
        dvecol = [35]
        dveflip = [0]
        late = [False]

        def mm_piece(q, rt, g, co, w):
            lhs = lhs_ap(q, rt)
            while w > 0:
                if st["tile"] is None:
                    st["tile"] = PS.tile(
                        [128, PSW], F32, tag="ps", name=f"mm{st['n']}"
                    )
                    st["n"] += 1
                # a matmul output cannot cross a 512-col PSUM bank boundary
                take = min(w, PSW - st["fill"], 512 - st["fill"] % 512)
                pe_order(nc.tensor.matmul(
                    out=st["tile"][:, st["fill"] : st["fill"] + take],
                    lhsT=lhs,
                    rhs=rhs_ap(g, co, take),
                    start=True,
                    stop=True,
                    perf_mode=DR,
                ))
                st["fill"] += take
                co += take
                w -= take
                if st["fill"] == PSW:
                    flush(q)

        def flush(q):
            if st["fill"]:
                dve_ok = late[0]
                use_dve = dve_ok and dveflip[0] % 2 == 0
                if dve_ok:
                    dveflip[0] += 1
                if use_dve:
                    # late-stage tile to the otherwise-idle DVE: bitcast exp
                    # (tensor_scalar affine -> i32, f32-bitcast reduce)
                    col = dvecol[0]
                    dvecol[0] += 1
                    w = st["fill"]
                    cv = W.tile([128, PSW], I32, tag="conv", name=f"cv{col}")
                    ts = nc.vector.tensor_scalar(
                        out=cv[:, 0:w], in0=st["tile"][:, 0:w],
                        scalar1=A_SCH, scalar2=B_SCH,
                        op0=op.mult, op1=op.add,
                    )
                    if last_dve[0] is not None:
                        add_dep_helper(ts.ins, last_dve[0].ins, sync=False,
                                       reason="dve order")
                    tr = nc.vector.tensor_reduce(
                        out=accw[:, col : col + 1],
                        in_=cv[:, 0:w].bitcast(F32),
                        axis=mybir.AxisListType.X,
                        op=op.add,
                    )
                    add_dep_helper(tr.ins, ts.ins, sync=False, reason="dve order")
                    last_dve[0] = tr
                else:
                    col = colctr[q]
                    colctr[q] += 1
                    act_order(nc.scalar.activation(
                        out=st["tile"][:, 0 : st["fill"]],
                        in_=st["tile"][:, 0 : st["fill"]],
                        func=mybir.ActivationFunctionType.Exp,
                        bias=bias_o[:],
                        scale=4.0,
                        accum_out=accw[:, col : col + 1],
                    ))
            st["tile"] = None
            st["fill"] = 0

        def self_stage(q):
            # 8 [128,128] self tiles of diag chunk q -> separate accum column
            t = PS.tile([128, PSW], F32, tag="ps", name=f"self{q}")
            for rt in range(8):
                pe_order(nc.tensor.matmul(
                    out=t[:, rt * 128 : (rt + 1) * 128],
                    lhsT=lhs_ap(q, rt),
                    rhs=lhs_ap(q, rt),
                    start=True,
                    stop=True,
                    perf_mode=DR,
                ))
            col = q * 17
            act_order(nc.scalar.activation(
                out=t[:, 0:1024],
                in_=t[:, 0:1024],
                func=mybir.ActivationFunctionType.Exp,
                bias=bias_o[:],
                scale=4.0,
                accum_out=accw[:, col : col + 1],
            ))

        def up_stage(q):
            # strict upper triangle of diag chunk q at 128-tile granularity
            for rt in range(8):
                s = (rt + 1) * 128
                for lo, hi in ((s, 512), (max(s, 512), 1024)):
                    if hi > lo:
                        mm_piece(q, rt, q * 2 + lo // 512, lo % 512, hi - lo)

        def o1_stage(q, cr):
            # diag chunk q rows vs full chunk cr (8192 cols = 4 ACTs)
            for rt in range(8):
                mm_piece(q, rt, cr * 2, 0, 512)
                mm_piece(q, rt, cr * 2 + 1, 0, 512)
            flush(q)

        def o2_stage(q):
            # distance-4 half block: rows rt vs one 512-row half of chunk q+4
            # (halves swapped for a1>=4 via the host-built gather order)
            for rt in range(8):
                g = (q + 4) * 2 + (0 if rt < 4 else 1)
                mm_piece(q, rt, g, 0, 512)
            flush(q)

        # ---- emission ----
        gather_chunk(0)
        setup_consts()
        for ch in range(1, 6):
            gather_chunk(ch)

        normalize(0, 8, "c0", True)
        transpose_chunk(0)
        self_stage(0)
        up_stage(0)
        normalize(8, 16, "c1", True)
        transpose_chunk(1)
        self_stage(1)
        up_stage(1)
        o1_stage(0, 1)
        normalize(16, 24, "c2", True)
        transpose_chunk(2)
        o1_stage(1, 2)
        o1_stage(0, 2)
        normalize(24, 32, "c3", True)
        transpose_chunk(3)
        o1_stage(1, 3)
        o1_stage(0, 3)
        normalize(32, 40, "c4", True)
        transpose_chunk(4)
        o1_stage(1, 4)
        o2_stage(0)
        normalize(40, 48, "c5", True)
        transpose_chunk(5)
        o2_stage(1)

        nc.sync.dma_start(out=acc, in_=accw[:])


def _build():
    nc = bacc.Bacc(
        "TRN2",
        target_bir_lowering=False,
        debug=False,
        enable_asserts=False,
        num_devices=NCORES,
    )
    tabs = nc.dram_tensor("tabs", [2 * NROWS, DIM], F32, kind="ExternalInput").ap()
    gidx = nc.dram_tensor("gidx", [128, NBAND], I32, kind="ExternalInput").ap()
    acc = nc.dram_tensor("acc", [128, ACC_W], F32, kind="ExternalOutput").ap()
    with tile.TileContext(nc) as tc:
        _body(tc, tabs, gidx, acc)
    nc.compile()
    return nc


_PROG = None


def _get_prog():
    global _PROG
    if _PROG is None:
        _PROG = _build()
    return _PROG


def _core_params(m):
    """core m -> (table t, first assignment a1)."""
    t = 0 if m < 4 else 1
    j = m % 4
    a1 = 2 * j + t  # u-cores: 0,2,4,6; p-cores: 1,3,5,7
    return t, a1


def _core_gidx(uid, pid, m):
    """[128, NBAND] int32 gather indices for core m (into the concat table)."""
    t, a1 = _core_params(m)
    main_ids = [uid, pid][t]
    ch = main_ids.reshape(NCORES, CHUNK)

    def h(a):  # quadrant half order for assignment a
        return 0 if a < 4 else 1

    segs = []
    for i in range(NCHUNK):
        cids = ch[(a1 + i) % NCORES].astype(np.int64) + t * NROWS
        if i == 4 and h(a1) == 1:
            cids = np.concatenate([cids[512:], cids[:512]])
        if i == 5 and h((a1 + 1) % NCORES) == 1:
            cids = np.concatenate([cids[512:], cids[:512]])
        segs.append(cids)
    slots = np.concatenate(segs).astype(np.int32)
    assert slots.shape == (NBAND * 128,)
    return np.ascontiguousarray(slots.reshape(NBAND, 128).T)


def _make_in_maps(user_id, pos_id, user_table, item_table):
    tabs = np.ascontiguousarray(
        np.concatenate(
            [
                np.asarray(user_table, dtype=np.float32),
                np.asarray(item_table, dtype=np.float32),
            ],
            axis=0,
        )
    )
    uid = np.asarray(user_id).astype(np.int64)
    pid = np.asarray(pos_id).astype(np.int64)
    return [
        {"tabs": tabs, "gidx": _core_gidx(uid, pid, m)} for m in range(NCORES)
    ]


def _host_align(user_id, pos_id, user_table, item_table):
    """align term in f64 on the host: 0.01% of the FLOPs, but 8 of 56 gather
    bands on the device's pacing GpSimd queue."""
    ue = np.asarray(user_table, dtype=np.float64)[np.asarray(user_id)]
    pe = np.asarray(item_table, dtype=np.float64)[np.asarray(pos_id)]
    un = ue / np.linalg.norm(ue, axis=1, keepdims=True)
    pn = pe / np.linalg.norm(pe, axis=1, keepdims=True)
    return 2.0 - (2.0 / B) * float(np.einsum("ij,ij->", un, pn))


def _finalize(accs, align):
    """accs: list of [128, ACC_W] per core -> scalar loss."""
    a = np.stack([np.asarray(x, dtype=np.float64) for x in accs])
    off_cols = [c for c in range(ACC_W) if c not in SELF_COLS]
    s_off_u = a[0:4][:, :, off_cols].sum()
    s_self_u = a[0:4][:, :, list(SELF_COLS)].sum()
    s_off_p = a[4:8][:, :, off_cols].sum()
    s_self_p = a[4:8][:, :, list(SELF_COLS)].sum()
    npairs = B * (B - 1) // 2
    pair_u = s_off_u + (s_self_u - B) / 2.0
    pair_p = s_off_p + (s_self_p - B) / 2.0
    unif = 0.5 * (np.log(pair_u / npairs) + np.log(pair_p / npairs))
    return np.asarray(align + unif, dtype=np.float32)


def _run(in_maps, trace=False, **kw):
    nc = _get_prog()
    return bass_utils.run_bass_kernel_spmd(
        nc, in_maps, core_ids=list(range(NCORES)), trace=trace, **kw
    )


def kernel(user_id, pos_id, neg_id=None, user_table=None, item_table=None):
    in_maps = _make_in_maps(user_id, pos_id, user_table, item_table)
    align = _host_align(user_id, pos_id, user_table, item_table)
    res = _run(in_maps, trace=False)
    return _finalize([res.results[m]["acc"] for m in range(NCORES)], align)


def _install_profile_hook():
    """The image's antenv lacks axon_hooks; shim it so trace=True can reach
    the NTFF profiler in libaxon_pjrt.so (same mechanism trn_boot uses)."""
    import sys
    import types

    if "antenv.axon_hooks" in sys.modules:
        return
    import antenv
    from trn_agent_boot.trn_boot import _ntff_profile_via_ctypes

    mod = types.ModuleType("antenv.axon_hooks")
    holder = [None]
    mod.set_axon_ntff_profile_hook = lambda h: holder.__setitem__(0, h)
    mod.get_axon_ntff_profile_hook = lambda: holder[0]
    sys.modules["antenv.axon_hooks"] = mod
    antenv.axon_hooks = mod
    mod.set_axon_ntff_profile_hook(
        _ntff_profile_via_ctypes("/opt/axon/libaxon_pjrt.so")
    )
    # no bucket filesystem in this container
    bass_utils.upload_artifacts = lambda tmpdir: ""


def run_profiled(user_id, pos_id, neg_id=None, user_table=None, item_table=None, **kw):
    _install_profile_hook()
    in_maps = _make_in_maps(user_id, pos_id, user_table, item_table)
    align = _host_align(user_id, pos_id, user_table, item_table)
    res = _run(in_maps, trace=True, **kw)
    out = _finalize([res.results[m]["acc"] for m in range(NCORES)], align)
    return out, res
